# revision 41
# baseline (speedup 1.0000x reference)
"""Trainium2 Bass kernel for nn_LuminaLM (4-layer GPT-2-like transformer + LM head).

Strategy: 8-way Megatron tensor parallel with sequence-parallel residual.
 - Each core owns 2 of 16 heads, 1/8 of the vocab; MLP is token-local
   (full fc weights streamed, bf16, host-precast).
 - Residual h is token-sharded: core r owns tokens [128r,128r+128) of each batch,
   stored feature-major as [128(dp), 8(dt), 128(t)] fp32 in SBUF.
 - All LN gains/biases are folded into the consuming weights ON THE HOST
   (ln1 -> w_qkv, ln2 -> w_fc1, lnf -> w_lm), so device LN is a pure
   (x-mean)*rstd normalize.
 - Weights are pre-cast to bf16 and pre-laid-out on the host so every weight
   DMA is a single contiguous HWDGE transfer (no cast-DMA, no gpsimd).
 - Per layer-half (half == batch): LN stats on shard (ones-matmul over d-tiles),
   normalize, AllGather bf16, qkv -> attention -> proj partial; both halves'
   partials go through ONE joint ReduceScatter -> residual add.
 - Attention: S = q@k^T per 128-query tile (causal, ragged), exp on ScalarE with
   accumulated row sums, P^T produced by a PE matmul against diag(1/sumexp)
   (fusing softmax normalization into the transpose), then y^T = v^T @ P^T.
 - LM head: vocab-sharded, bf16 weights streamed, logits fp32 out.
Matmuls are bf16 with fp32 PSUM accumulation; collectives ride bf16.
"""

import os
import numpy as np

B, T, D, V, L = 2, 1024, 1024, 32000, 4
H, HD = 16, 64
NCORES = 8
P = 128
TPC = T // NCORES          # 128 tokens per core per batch
HPC = H // NCORES          # 2 heads per core
QKVC = 3 * P               # 384 qkv cols per core (q:128, k:128, v:128)
VPC = V // NCORES          # 4000 vocab per core
MC = 125                   # lm-head M chunk (32 chunks of 125 = 4000)
NMC = VPC // MC            # 32
DT = D // P                # 8 d-tiles
NFC = 4 * D // P           # 32 fc1-output chunks
EPS = 1e-5
ATT_SCALE = 1.0 / np.sqrt(HD)

_CACHE = {}
last_exec_time_ns = None
last_res = None


def _build_nc(no_coll=False):
    import concourse.bass as bass
    import concourse.mybir as mybir
    import concourse.tile as tile
    from concourse import bacc
    from concourse.masks import make_identity
    from concourse.bass import IndirectOffsetOnAxis

    dt = mybir.dt
    AF = mybir.ActivationFunctionType
    OP = mybir.AluOpType

    nc = bacc.Bacc("TRN2", target_bir_lowering=False, debug=False,
                   num_devices=NCORES)

    # ---- external parameters (per-core shards, staged by host) ----
    ids_p = nc.declare_dram_parameter("ids_sh", [TPC, B], dt.int32, isOutput=False)
    wte_p = nc.declare_dram_parameter("wte", [V, D], dt.float32, isOutput=False)
    wpe_p = nc.declare_dram_parameter("wpe_sh", [TPC, D], dt.float32, isOutput=False)
    wqkv_p = nc.declare_dram_parameter("wqkv_sh", [L, P, DT, QKVC], dt.bfloat16, isOutput=False)
    bqkv_p = nc.declare_dram_parameter("bqkv_sh", [P, L, 3], dt.float32, isOutput=False)
    wproj_p = nc.declare_dram_parameter("wproj_sh", [L, P, D], dt.bfloat16, isOutput=False)
    bproj_p = nc.declare_dram_parameter("bproj", [P, L, DT], dt.float32, isOutput=False)
    wfc1_p = nc.declare_dram_parameter("wfc1", [L, NFC, P, DT, P], dt.bfloat16, isOutput=False)
    bfc1_p = nc.declare_dram_parameter("bfc1", [P, L, NFC], dt.float32, isOutput=False)
    wfc2_p = nc.declare_dram_parameter("wfc2", [L, DT, P, NFC, P], dt.bfloat16, isOutput=False)
    bfc2_p = nc.declare_dram_parameter("bfc2", [P, L, DT], dt.float32, isOutput=False)
    wlm_p = nc.declare_dram_parameter("wlm_sh", [NMC, P, DT, MC], dt.bfloat16, isOutput=False)
    blm_p = nc.declare_dram_parameter("blm_sh", [MC, NMC], dt.float32, isOutput=False)
    logits_p = nc.declare_dram_parameter("logits", [VPC, B * T], dt.float32, isOutput=True)

    RG = [list(range(NCORES))]

    with tile.TileContext(nc) as tc:
        with (
            tc.tile_pool(name="const", bufs=1) as cp,
            tc.tile_pool(name="wp", bufs=2) as wp,
            tc.tile_pool(name="ap", bufs=2) as app,
            tc.tile_pool(name="psA", bufs=2, space="PSUM") as psA,
            tc.tile_pool(name="psS", bufs=4, space="PSUM") as psS,
            tc.tile_pool(name="psB", bufs=2, space="PSUM") as psB,
            tc.tile_pool(name="dram", bufs=2, space="DRAM") as dramp,
        ):
            # ---------------- constants ----------------
            ident_bf = cp.tile([P, P], dt.bfloat16)
            make_identity(nc, ident_bf[:])
            ident_f = cp.tile([P, P], dt.float32)
            make_identity(nc, ident_f[:])
            ones_col_bf = cp.tile([P, 1], dt.bfloat16)
            nc.any.memset(ones_col_bf[:], 1.0)
            ones_row_f = cp.tile([1, P], dt.float32)
            nc.any.memset(ones_row_f[:], 1.0)
            # transposed causal masks over a 256-query pair block:
            # cmask_a[k, q] = 0 where q >= k      (diagonal kt == 2*qp)
            # cmask_b[k, q] = 0 where q >= k+128  (diagonal kt == 2*qp+1)
            cmask_a = cp.tile([P, 2 * P], dt.float32)
            nc.gpsimd.memset(cmask_a[:], 0.0)
            nc.gpsimd.affine_select(
                out=cmask_a[:], in_=cmask_a[:], compare_op=OP.is_ge,
                fill=-1e9, base=0, pattern=[[1, 2 * P]], channel_multiplier=-1,
            )
            cmask_b = cp.tile([P, 2 * P], dt.float32)
            nc.gpsimd.memset(cmask_b[:], 0.0)
            nc.gpsimd.affine_select(
                out=cmask_b[:], in_=cmask_b[:], compare_op=OP.is_ge,
                fill=-1e9, base=-P, pattern=[[1, 2 * P]], channel_multiplier=-1,
            )

            # biases (single contiguous DMAs)
            bqkv_all = cp.tile([P, L, 3], dt.float32)
            nc.sync.dma_start(bqkv_all[:], bqkv_p[:])
            bproj_all = cp.tile([P, L, DT], dt.float32)
            nc.sync.dma_start(bproj_all[:], bproj_p[:])
            bfc1_all = cp.tile([P, L, NFC], dt.float32)
            nc.sync.dma_start(bfc1_all[:], bfc1_p[:])
            bfc2_all = cp.tile([P, L, DT], dt.float32)
            nc.sync.dma_start(bfc2_all[:], bfc2_p[:])
            blm_all = cp.tile([MC, NMC], dt.float32)
            nc.sync.dma_start(blm_all[:], blm_p[:])

            # wpe [128 tok, D] fp32 (shares the mTm slot: dead after embedding);
            # token indices [128, B] int32
            wpe_tok = app.tile([TPC, D], dt.float32, name="wpe_tok", tag="mTm",
                               bufs=1)
            nc.sync.dma_start(wpe_tok[:], wpe_p[:])
            idx_sb = cp.tile([TPC, B], dt.int32)
            nc.sync.dma_start(idx_sb[:], ids_p[:])

            # ---------------- embedding ----------------
            # residual shard per half: [128(dp), DT, 128(t)] fp32 (persistent)
            hres = [cp.tile([P, DT, TPC], dt.float32, name=f"hres{h}") for h in range(B)]
            with nc.named_scope("embed"):
                for half in range(B):
                    emb = app.tile([TPC, D], dt.float32, name="emb", tag="emb", bufs=1)
                    nc.gpsimd.indirect_dma_start(
                        out=emb[:], out_offset=None, in_=wte_p[:],
                        in_offset=IndirectOffsetOnAxis(ap=idx_sb[:, half:half + 1], axis=0),
                    )
                    nc.vector.tensor_add(emb[:], emb[:], wpe_tok[:])
                    for dti in range(DT):
                        pst = psB.tile([P, P], dt.float32, space="PSUM", name="pst_emb",
                                       tag="psB")
                        nc.tensor.transpose(pst[:], emb[:, dti * P:(dti + 1) * P], ident_f[:])
                        nc.vector.tensor_copy(hres[half][:, dti, :], pst[:])

            # ---------------- helpers ----------------
            def ln_stats(h_tile, name):
                """Returns (hb, rstd_full, mrstd_full); hb is the bf16 copy of
                h_tile, rstd/mrstd are [128, TPC] fp32 broadcast tiles."""
                hb = app.tile([P, DT, TPC], dt.bfloat16, name=f"hb_{name}", tag="hb")
                nc.vector.tensor_copy(hb[:], h_tile[:])
                hb2 = app.tile([P, DT, TPC], dt.bfloat16, name=f"hb2_{name}", tag="hb2")
                nc.vector.tensor_mul(hb2[:], hb[:], hb[:])
                ps_sum = psB.tile([1, TPC], dt.float32, space="PSUM", name=f"psum_{name}", tag="psB")
                ps_sq = psB.tile([1, TPC], dt.float32, space="PSUM", name=f"psq_{name}", tag="psB")
                for dti in range(DT):
                    nc.tensor.matmul(ps_sum[:], lhsT=ones_col_bf[:], rhs=hb[:, dti, :],
                                     start=(dti == 0), stop=(dti == DT - 1))
                for dti in range(DT):
                    nc.tensor.matmul(ps_sq[:], lhsT=ones_col_bf[:], rhs=hb2[:, dti, :],
                                     start=(dti == 0), stop=(dti == DT - 1))
                m_sb = app.tile([1, TPC], dt.float32, name=f"m_{name}", tag="m")
                nc.vector.tensor_scalar_mul(m_sb[:], ps_sum[:], 1.0 / D)
                # var = sq/D - m*m + eps ; rstd = 1/sqrt(var)
                mm_sb = app.tile([1, TPC], dt.float32, name=f"mm_{name}", tag="mm")
                nc.vector.tensor_mul(mm_sb[:], m_sb[:], m_sb[:])
                var_sb = app.tile([1, TPC], dt.float32, name=f"var_{name}", tag="var")
                nc.vector.scalar_tensor_tensor(
                    out=var_sb[:], in0=ps_sq[:], scalar=1.0 / D, in1=mm_sb[:],
                    op0=OP.mult, op1=OP.subtract)
                nc.vector.tensor_scalar_add(var_sb[:], var_sb[:], EPS)
                std_sb = app.tile([1, TPC], dt.float32, name=f"std_{name}", tag="std")
                nc.scalar.activation(std_sb[:], var_sb[:], AF.Sqrt)
                rstd_sb = app.tile([1, TPC], dt.float32, name=f"rstd_{name}", tag="rstd")
                nc.vector.reciprocal(rstd_sb[:], std_sb[:])
                mrstd_sb = app.tile([1, TPC], dt.float32, name=f"mrstd_{name}", tag="mrstd")
                nc.vector.scalar_tensor_tensor(
                    out=mrstd_sb[:], in0=m_sb[:], scalar=-1.0, in1=rstd_sb[:],
                    op0=OP.mult, op1=OP.mult)
                # broadcast across partitions via K=1 fp32 matmuls
                ps_r = psB.tile([P, TPC], dt.float32, space="PSUM", name=f"psr_{name}", tag="psB")
                nc.tensor.matmul(ps_r[:], lhsT=ones_row_f[:], rhs=rstd_sb[:],
                                 start=True, stop=True)
                rstd_full = app.tile([P, TPC], dt.float32, name=f"rstdf_{name}", tag="rstdf")
                nc.vector.tensor_copy(rstd_full[:], ps_r[:])
                ps_mr = psB.tile([P, TPC], dt.float32, space="PSUM", name=f"psmr_{name}", tag="psB")
                nc.tensor.matmul(ps_mr[:], lhsT=ones_row_f[:], rhs=mrstd_sb[:],
                                 start=True, stop=True)
                mrstd_full = app.tile([P, TPC], dt.float32, name=f"mrstdf_{name}", tag="mrstdf")
                nc.vector.tensor_copy(mrstd_full[:], ps_mr[:])
                return hb, rstd_full, mrstd_full

            def normalize(h_tile, out_tile, out_off, name):
                """Pure LN normalize (h-m)*rstd of the token shard into
                out_tile[:, :, out_off:out_off+TPC] (bf16)."""
                _, rstd_full, mrstd_full = ln_stats(h_tile, name)
                t1 = app.tile([P, DT, TPC], dt.bfloat16, name=f"t1_{name}", tag="t1")
                nc.vector.tensor_tensor(
                    out=t1[:], in0=h_tile[:],
                    in1=rstd_full[:, None, :].to_broadcast([P, DT, TPC]), op=OP.mult)
                nc.vector.tensor_tensor(
                    out=out_tile[:, :, out_off:out_off + TPC], in0=t1[:],
                    in1=mrstd_full[:, None, :].to_broadcast([P, DT, TPC]), op=OP.add)

            def layernorm_to_bounce(h_tile, name):
                """LN on the token shard -> DRAM bounce [D, TPC] bf16."""
                hn = app.tile([P, DT, TPC], dt.bfloat16, name=f"hn_{name}", tag="hn")
                normalize(h_tile, hn, 0, name)
                ag_in = dramp.tile([D, TPC], dt.bfloat16, name=f"agin_{name}", tag="agin")
                nc.sync.dma_start(ag_in[:].rearrange("(dt p) t -> p dt t", p=P), hn[:])
                return ag_in

            def allgather_read(ag_in, name):
                """AllGather the shard; read back as [128, DT, T] bf16."""
                ag_out = dramp.tile([NCORES * D, TPC], dt.bfloat16,
                                    name=f"agout_{name}", tag="agout",
                                    addr_space="Shared")
                if no_coll:
                    nc.sync.dma_start(ag_out[0:D, :], ag_in[:])
                else:
                    nc.gpsimd.collective_compute(
                        "AllGather", OP.bypass, replica_groups=RG,
                        ins=[ag_in[:].opt()], outs=[ag_out[:].opt()],
                    )
                aT = app.tile([P, DT, NCORES, TPC], dt.bfloat16, name=f"aT_{name}",
                              tag="aT", bufs=1)
                ag_view = ag_out[:].rearrange("(r dt p) t -> p dt r t", p=P, dt=DT)
                for dti in range(DT):
                    nc.sync.dma_start(aT[:, dti, :, :], ag_view[:, dti, :, :])
                return aT.rearrange("p dt r t -> p dt (r t)")

            def load_weights(li):
                wqkv = wp.tile([P, DT, QKVC], dt.bfloat16, name=f"wqkv{li}", tag="wqkv")
                nc.sync.dma_start(wqkv[:], wqkv_p[li])
                wproj = wp.tile([P, D], dt.bfloat16, name=f"wproj{li}", tag="wproj")
                nc.sync.dma_start(wproj[:], wproj_p[li])
                return wqkv, wproj

            NT = T // 512  # 2 token chunks of 512 per half

            def qkv_block(aT, wqkv, li, half):
                qkvT = app.tile([P, 3, T], dt.bfloat16, name=f"qkvT{half}", tag="qkvT")
                for c in (2, 1, 0):  # v first: unblocks the v transposes early
                    for tk in range(NT):
                        ps = psA.tile([P, 512], dt.float32, space="PSUM", name="ps_qkv", tag="psA")
                        for dti in range(DT):
                            nc.tensor.matmul(
                                ps[:], lhsT=wqkv[:, dti, c * P:(c + 1) * P],
                                rhs=aT[:, dti, tk * 512:(tk + 1) * 512],
                                start=(dti == 0), stop=(dti == DT - 1))
                        nc.vector.tensor_scalar_add(
                            qkvT[:, c, tk * 512:(tk + 1) * 512], ps[:],
                            bqkv_all[:, li, c:c + 1])
                return qkvT

            HDA = HD + 1  # v columns per head, augmented with a ones column

            def attn_setup(qkvT, half):
                """v transposed to token-major [128(t), 8(tt), 130], where cols
                [h2*65, h2*65+64) are head h2's v features and col h2*65+64
                is 1.0 (so AV also produces the softmax denominator row)."""
                v_tok = app.tile([P, DT, HPC * HDA], dt.bfloat16,
                                 name=f"vtok{half}", tag="vtok")
                nc.any.memset(v_tok[:, :, HD:HD + 1], 1.0)
                nc.any.memset(v_tok[:, :, HDA + HD:HDA + HD + 1], 1.0)
                for tt in range(DT):
                    pst = psB.tile([P, P], dt.bfloat16, space="PSUM", name="pst_v",
                                   tag="psB")
                    nc.tensor.transpose(
                        pst[:], qkvT[:, 2, tt * P:(tt + 1) * P], ident_bf[:])
                    nc.vector.tensor_copy(v_tok[:, tt, 0:HD], pst[:, 0:HD])
                    nc.vector.tensor_copy(v_tok[:, tt, HDA:HDA + HD], pst[:, HD:P])
                yT = app.tile([P, T], dt.bfloat16, name=f"yT{half}", tag="yT")
                return v_tok, yT

            def attention_chunk(qkvT, v_tok, yT, half, qc):
                """S^T + exp + AV for one 512-query chunk of one half."""
                if True:
                    PT = [app.tile([P, DT, 512], dt.bfloat16,
                                   name=f"PT{half}_{qc}_{h2}", tag="PT", bufs=3)
                          for h2 in range(HPC)]
                    for qp in range(qc * 2, qc * 2 + 2):  # 256-query pair blocks
                        qoff = (qp - qc * 2) * 2 * P
                        nkt = 2 * qp + 2
                        for h2 in range(HPC):
                            hs = h2 * HD
                            q_sl = qkvT[hs:hs + HD, 0, qp * 2 * P:(qp + 1) * 2 * P]
                            # S^T tiles: one N=256 matmul per 128-key tile; 2
                            # tiles share a psum group, exp'd (with the causal
                            # masks on the two diagonal tiles) straight into PT.
                            for kt0 in range(0, nkt, 2):
                                ng = min(2, nkt - kt0)
                                ps_st = psS.tile([P, 512], dt.float32, space="PSUM",
                                                 name="ps_st", tag="psS")
                                for j in range(ng):
                                    kt = kt0 + j
                                    nc.tensor.matmul(
                                        ps_st[:, j * 2 * P:(j + 1) * 2 * P],
                                        lhsT=qkvT[hs:hs + HD, 1, kt * P:(kt + 1) * P],
                                        rhs=q_sl, start=True, stop=True)
                                    if kt == 2 * qp:
                                        nc.vector.tensor_add(
                                            ps_st[:, j * 2 * P:(j + 1) * 2 * P],
                                            ps_st[:, j * 2 * P:(j + 1) * 2 * P],
                                            cmask_a[:])
                                    elif kt == 2 * qp + 1:
                                        nc.vector.tensor_add(
                                            ps_st[:, j * 2 * P:(j + 1) * 2 * P],
                                            ps_st[:, j * 2 * P:(j + 1) * 2 * P],
                                            cmask_b[:])
                                nc.scalar.activation(
                                    PT[h2][:, kt0:kt0 + ng, qoff:qoff + 2 * P],
                                    ps_st[:, :ng * 2 * P].rearrange(
                                        "p (k q) -> p k q", q=2 * P),
                                    AF.Exp, scale=ATT_SCALE)
                    # AV for this 512-query chunk (augmented v -> row 64 = sum)
                    nkt = qc * 4 + 4
                    for h2 in range(HPC):
                        ps_y = psS.tile([HDA, 512], dt.float32, space="PSUM",
                                        name=f"ps_y{h2}", tag="psS")
                        for kt in range(nkt):
                            qstart = max(kt * P, qc * 512)
                            off = qstart - qc * 512
                            nc.tensor.matmul(
                                ps_y[:, off:512],
                                lhsT=v_tok[:, kt, h2 * HDA:(h2 + 1) * HDA],
                                rhs=PT[h2][:, kt, off:512],
                                start=(kt == 0), stop=(kt == nkt - 1))
                        yraw = app.tile([HDA, 512], dt.float32,
                                        name=f"yraw{h2}", tag="yraw")
                        nc.vector.tensor_copy(yraw[:], ps_y[:])
                        se_t = app.tile([1, 512], dt.float32, name="se_t", tag="se_t")
                        nc.vector.tensor_copy(se_t[:], yraw[HD:HDA, :])
                        rec = app.tile([1, 512], dt.float32, name="rec", tag="rec")
                        rscr = app.tile([1, 512], dt.float32, name="rscr", tag="rscr")
                        nc.vector.reciprocal_approx_accurate(rec[:], se_t[:], rscr[:])
                        recb = app.tile([HD, 512], dt.float32, name="recb",
                                        tag="recb")
                        nc.gpsimd.partition_broadcast(recb[:], rec[:])
                        # inputs at base partition 0; only the output shifts
                        nc.vector.tensor_tensor(
                            out=yT[h2 * HD:(h2 + 1) * HD, qc * 512:(qc + 1) * 512],
                            in0=yraw[0:HD, :], in1=recb[:], op=OP.mult)

            def proj_partial(yT, wproj, half):
                """Compute this half's proj partial into an RS input buffer,
                blocked by destination token block. Returns the buffer."""
                rs_in = dramp.tile([NCORES * D, TPC], dt.bfloat16,
                                   name=f"rsin{half}", tag=f"rsin{half}")
                rs_view = rs_in[:].rearrange(
                    "(tb dc p) tw -> p dc tb tw", p=P, dc=DT)
                for dc in range(DT):
                    for tk in range(NT):
                        ps = psA.tile([P, 512], dt.float32, space="PSUM", name="ps_pr", tag="psA")
                        nc.tensor.matmul(
                            ps[:], lhsT=wproj[:, dc * P:(dc + 1) * P],
                            rhs=yT[:, tk * 512:(tk + 1) * 512], start=True, stop=True)
                        prc = app.tile([P, 512], dt.bfloat16, name="prc", tag="prc",
                                       bufs=3)
                        nc.vector.tensor_copy(prc[:], ps[:])
                        nc.sync.dma_start(
                            rs_view[:, dc, tk * 4:(tk + 1) * 4, :],
                            prc[:].rearrange("p (tb tw) -> p tb tw", tw=TPC))
                return rs_in

            def reduce_scatter(rs_in, h):
                """Per-half RS collective only (no readback)."""
                rs_out = dramp.tile([D, TPC], dt.bfloat16, name=f"rsout{h}",
                                    tag=f"rsout{h}")
                if no_coll:
                    nc.sync.dma_start(rs_out[:], rs_in[0:D, :])
                else:
                    nc.gpsimd.collective_compute(
                        "ReduceScatter", OP.add, replica_groups=RG,
                        ins=[rs_in[:].opt()], outs=[rs_out[:].opt()],
                    )
                return rs_out

            def residual_from_rs(rs_out, li, h):
                rsb = app.tile([P, DT, TPC], dt.bfloat16, name=f"rsb{h}", tag="rsb")
                nc.sync.dma_start(rsb[:], rs_out[:].rearrange(
                    "(dc p) tw -> p dc tw", p=P))
                for dc in range(DT):
                    nc.vector.scalar_tensor_tensor(
                        out=hres[h][:, dc, :], in0=rsb[:, dc, :],
                        scalar=bproj_all[:, li, dc:dc + 1], in1=hres[h][:, dc, :],
                        op0=OP.add, op1=OP.add)

            def mlp_fc1(hn2m, mTm, li):
                """fc1+gelu over both halves (256 tokens), streamed chunks."""
                for fc in range(NFC):
                    wf1c = wp.tile([P, DT, P], dt.bfloat16, name=f"wf1c{li}_{fc}",
                                   tag="wf1c", bufs=6)
                    nc.sync.dma_start(wf1c[:], wfc1_p[li, fc])
                    ps = psA.tile([P, B * TPC], dt.float32, space="PSUM",
                                  name="ps_f1", tag="psA")
                    for dti in range(DT):
                        nc.tensor.matmul(
                            ps[:], lhsT=wf1c[:, dti, :], rhs=hn2m[:, dti, :],
                            start=(dti == 0), stop=(dti == DT - 1))
                    nc.scalar.activation(
                        mTm[:, fc, :], ps[:], AF.Gelu,
                        bias=bfc1_all[:, li, fc:fc + 1])

            def mlp_fc2(mTm, li):
                """fc2 over both halves (256 tokens), streamed chunks."""
                for dc in range(DT):
                    wf2c = wp.tile([P, NFC, P], dt.bfloat16, name=f"wf2c{li}_{dc}",
                                   tag="wf2c", bufs=2)
                    nc.sync.dma_start(wf2c[:], wfc2_p[li, dc])
                    ps2 = psA.tile([P, B * TPC], dt.float32, space="PSUM",
                                   name="ps_f2", tag="psA")
                    for kt in range(NFC):
                        nc.tensor.matmul(
                            ps2[:], lhsT=wf2c[:, kt, :], rhs=mTm[:, kt, :],
                            start=(kt == 0), stop=(kt == NFC - 1))
                    for h in range(B):
                        nc.vector.scalar_tensor_tensor(
                            out=hres[h][:, dc, :],
                            in0=ps2[:, h * TPC:(h + 1) * TPC],
                            scalar=bfc2_all[:, li, dc:dc + 1],
                            in1=hres[h][:, dc, :], op0=OP.add, op1=OP.add)

            # ---------------- transformer layers ----------------
            for li in range(L):
                with nc.named_scope(f"L{li}"):
                    wqkv, wproj = load_weights(li)
                    ag1 = [layernorm_to_bounce(hres[h], f"l{li}a{h}") for h in range(B)]
                    hn2m = app.tile([P, DT, B * TPC], dt.bfloat16, name=f"hn2m{li}",
                                    tag="hn2m", bufs=1)
                    mTm = app.tile([P, NFC, B * TPC], dt.bfloat16, name=f"mTm{li}",
                                   tag="mTm", bufs=1)
                    qkvTs, vtoks, yTs = [], [], []
                    for h in range(B):
                        aT = allgather_read(ag1[h], f"l{li}a{h}")
                        qkvTs.append(qkv_block(aT, wqkv, li, h))
                        vt, yt = attn_setup(qkvTs[h], h)
                        vtoks.append(vt)
                        yTs.append(yt)
                    # interleave the halves' chunks so each half's ACT/DVE
                    # latency hides under the other half's PE work; h0 still
                    # finishes first so its RS hides under h1's tail.
                    attention_chunk(qkvTs[0], vtoks[0], yTs[0], 0, 0)
                    attention_chunk(qkvTs[1], vtoks[1], yTs[1], 1, 0)
                    attention_chunk(qkvTs[0], vtoks[0], yTs[0], 0, 1)
                    rs_outs = [reduce_scatter(proj_partial(yTs[0], wproj, 0), 0)]
                    attention_chunk(qkvTs[1], vtoks[1], yTs[1], 1, 1)
                    rs_outs.append(reduce_scatter(proj_partial(yTs[1], wproj, 1), 1))
                    for h in range(B):
                        residual_from_rs(rs_outs[h], li, h)
                        normalize(hres[h], hn2m, h * TPC, f"l{li}m{h}")
                    mlp_fc1(hn2m, mTm, li)
                    mlp_fc2(mTm, li)

            # ---------------- final LN + LM head ----------------
            with nc.named_scope("lmhead"):
                agf = [layernorm_to_bounce(hres[h], f"f{h}") for h in range(B)]
                for h in range(B):
                    afT = allgather_read(agf[h], f"f{h}")
                    for mc in range(NMC):
                        wlm = app.tile([P, DT, MC], dt.bfloat16,
                                       name=f"wlm{h}_{mc}", tag="wlm", bufs=3)
                        nc.sync.dma_start(wlm[:], wlm_p[mc])
                        for tk in range(NT):
                            ps = psA.tile([P, 512], dt.float32, space="PSUM", name="ps_lm", tag="psA")
                            for dti in range(DT):
                                nc.tensor.matmul(
                                    ps[:MC, :], lhsT=wlm[:, dti, :],
                                    rhs=afT[:, dti, tk * 512:(tk + 1) * 512],
                                    start=(dti == 0), stop=(dti == DT - 1))
                            lsb = app.tile([MC, 512], dt.float32, name="lsb", tag="lsb",
                                           bufs=2)
                            nc.vector.tensor_scalar_add(lsb[:], ps[:MC, :],
                                                        blm_all[:, mc:mc + 1])
                            nc.sync.dma_start(
                                logits_p[mc * MC:(mc + 1) * MC,
                                         h * T + tk * 512:h * T + (tk + 1) * 512],
                                lsb[:])

    nc.compile()
    return nc


def _get_nc():
    no_coll = os.environ.get("KERNEL_NO_COLL", "0") == "1"
    key = ("nc", no_coll)
    if key not in _CACHE:
        _CACHE[key] = _build_nc(no_coll)
    return _CACHE[key]


def build_in_maps(input_ids, wte, wpe, ln1_g, ln1_b, w_qkv, b_qkv, w_proj,
                  b_proj, ln2_g, ln2_b, w_fc1, b_fc1, w_fc2, b_fc2, lnf_g,
                  lnf_b, w_lm):
    from ml_dtypes import bfloat16
    f32 = np.float32

    def ca(x):
        return np.ascontiguousarray(x)

    ids = np.asarray(input_ids).astype(np.int32)
    wte = np.asarray(wte, f32)
    wpe = np.asarray(wpe, f32)
    w_qkv = np.asarray(w_qkv, f32)
    b_qkv = np.asarray(b_qkv, f32)
    w_proj = np.asarray(w_proj, f32)
    b_proj = np.asarray(b_proj, f32)
    w_fc1 = np.asarray(w_fc1, f32)
    b_fc1 = np.asarray(b_fc1, f32)
    w_fc2 = np.asarray(w_fc2, f32)
    b_fc2 = np.asarray(b_fc2, f32)

    # ---- fold LN gains/biases into consuming weights (host, fp32) ----
    # ln1 -> w_qkv : qkv = hn @ (g1*W) + (b1 @ (g1*W) + b)
    wqkv_eff = w_qkv * np.asarray(ln1_g, f32)[:, :, None]          # [L, D, 3D]
    bqkv_eff = np.einsum('ld,ldf->lf', np.asarray(ln1_b, f32), wqkv_eff) + b_qkv
    # ln2 -> w_fc1
    wfc1_eff = w_fc1 * np.asarray(ln2_g, f32)[:, :, None]          # [L, D, 4D]
    bfc1_eff = np.einsum('ld,ldf->lf', np.asarray(ln2_b, f32), wfc1_eff) + b_fc1
    # lnf -> w_lm
    wlm_eff = np.asarray(w_lm, f32) * np.asarray(lnf_g, f32)[:, None]   # [D, V]
    blm_eff = np.asarray(lnf_b, f32) @ wlm_eff                     # [V]

    # host layouts (see declare_dram_parameter shapes)
    wfc1_host = ca(wfc1_eff.reshape(L, DT, P, NFC, P).transpose(0, 3, 2, 1, 4)
                   .astype(bfloat16))                               # [L,NFC,P,DT,P]
    bfc1_host = ca(bfc1_eff.reshape(L, NFC, P).transpose(2, 0, 1))  # [P,L,NFC]
    wfc2_host = ca(w_fc2.reshape(L, NFC, P, DT, P).transpose(0, 3, 2, 1, 4)
                   .astype(bfloat16))                               # [L,DT,P,NFC,P]
    bfc2_host = ca(b_fc2.reshape(L, DT, P).transpose(2, 0, 1))      # [P,L,DT]
    bproj_host = ca(b_proj.reshape(L, DT, P).transpose(2, 0, 1))    # [P,L,DT]

    in_maps = []
    for r in range(NCORES):
        t0, t1 = r * TPC, (r + 1) * TPC
        cols = np.r_[P * r:P * r + P, D + P * r:D + P * r + P,
                     2 * D + P * r:2 * D + P * r + P]
        vs, ve = r * VPC, (r + 1) * VPC
        wqkv_sh = ca(wqkv_eff[:, :, cols].reshape(L, DT, P, QKVC)
                     .transpose(0, 2, 1, 3).astype(bfloat16))       # [L,P,DT,3P]
        bqkv_sh = ca(bqkv_eff[:, cols].reshape(L, 3, P).transpose(2, 0, 1))  # [P,L,3]
        wlm_sh = ca(wlm_eff[:, vs:ve].reshape(DT, P, NMC, MC)
                    .transpose(2, 1, 0, 3).astype(bfloat16))        # [NMC,P,DT,MC]
        blm_sh = ca(blm_eff[vs:ve].reshape(NMC, MC).T)              # [MC,NMC]
        in_maps.append({
            "ids_sh": ca(ids[:, t0:t1].T),                          # [TPC,B]
            "wte": wte,
            "wpe_sh": ca(wpe[t0:t1]),
            "wqkv_sh": wqkv_sh,
            "bqkv_sh": bqkv_sh,
            "wproj_sh": ca(w_proj[:, P * r:P * r + P, :].astype(bfloat16)),
            "bproj": bproj_host,
            "wfc1": wfc1_host,
            "bfc1": bfc1_host,
            "wfc2": wfc2_host,
            "bfc2": bfc2_host,
            "wlm_sh": wlm_sh,
            "blm_sh": blm_sh,
        })

    return in_maps


def kernel(**inputs):
    global last_exec_time_ns, last_res
    from concourse.bass_utils import run_bass_kernel_spmd

    in_maps = build_in_maps(**inputs)
    nc = _get_nc()
    trace = os.environ.get("KERNEL_TRACE", "0") == "1"
    res = run_bass_kernel_spmd(nc, in_maps, list(range(NCORES)), trace=trace)
    last_exec_time_ns = res.exec_time_ns
    last_res = res

    parts = [res.results[r]["logits"] for r in range(NCORES)]  # [VPC, B*T] each
    full = np.concatenate(parts, axis=0)          # [V, B*T]
    out = full.T.reshape(B, T, V).astype(np.float32)
    return out
